# revision 31
# baseline (speedup 1.0000x reference)
"""GNN message-passing layer (normalized-adjacency conv + linear + LeakyReLU)
on 8 Trainium2 NeuronCores, pure data parallel over the batch dim.

Computation (per batch b):
    deg      = adj.sum(-1)                     # [N]
    agg      = (adj / deg[:, None]) @ X        # [N, FIN]
    out      = leakyrelu(agg @ W.T + bias)     # [N, FOUT]

Device-side formulation. adj is host-transposed per batch (adjT[k, m] =
adj[m, k]) so the contraction index k sits on SBUF partitions for both matmul
operands, and everything downstream stays transposed ([feature, node] order)
so all PE work streams 512-wide:
    rawT[f, m]   = sum_k X[k, f] * adjT[k, m]    # X tiles as weights
    degbc[:, m]  = sum_k 1 * adjT[k, m]          # ones weights -> deg
                                                 # broadcast to all partitions
    out2T[o, m]  = sum_f WT[f, o] * rawT[f, m]   # W as weights
    t            = out2T * (1/degbc)             # DVE multiply
    outT[o, m]   = Lrelu(t + b)                  # scalar engine, per-partition b
The DRAM output is [B, FOUT, N] fp16; the host swaps the last two axes and
casts to fp32.

Everything DMA'd is fp16 (half the HBM traffic of fp32; adj/X values are
well inside fp16 range and the 2^-11 rounding is far below the accuracy
gate). The deg reduction over the 8 k-tiles is split: 4 pairwise adds on the
DVE (fp16, 2x mode) fold 8 tiles to 4, then a 4-matmul PSUM accumulation
with ones weights folds the rest and broadcasts deg to all 128 partitions.
"""

import numpy as np

import concourse.bass as bass
import concourse.mybir as mybir
import concourse.tile as tile
from concourse.bass_utils import run_bass_kernel_spmd

P = 128

# Problem shape (hardcoded per the harness contract).
B, N, FIN, FOUT = 32, 1024, 128, 128
NEG_SLOPE = 0.01
N_CORES = 8
BPC = B // N_CORES  # batches per core

USE_LRELU = False


def build_bass(nbatch=BPC, n=N, fin=FIN, fout=FOUT, neg_slope=NEG_SLOPE,
               adj_bufs=8, use_lrelu=USE_LRELU):
    f32 = mybir.dt.float32
    f16 = mybir.dt.float16
    alpha = float(neg_slope)
    nc = bass.Bass()

    KT = n // P          # contraction tiles (8)
    CH = min(512, n)     # matmul moving free dim (one fp32 PSUM bank)
    NCH = n // CH        # moving-dim chunks (2)
    NAC = 4              # adj DMA chunks per batch (512 KiB each)
    KG = KT // NAC       # k-tiles per adj DMA chunk (2)

    # All DRAM layouts are host-staged so every DMA is one fully linear
    # run per partition (128 descriptors/transfer instead of 512+): the
    # Sync-engine HWDGE dispatch cost is descriptor-bound.
    adjT = nc.dram_tensor("adjT", [nbatch, NAC, P, KG, n], f16,
                          kind="ExternalInput")
    x = nc.dram_tensor("x", [P, nbatch, n // P, fin], f16,
                       kind="ExternalInput")
    w2 = nc.dram_tensor("w2", [P, 2, P], f16, kind="ExternalInput")
    bvec = nc.dram_tensor("bvec", [P, 1], f32, kind="ExternalInput")
    outT = nc.dram_tensor("outT", [nbatch, fout, n], f16, kind="ExternalOutput")

    with tile.TileContext(nc) as tc:
        with (
            tc.tile_pool(name="const", bufs=1) as cpool,
            tc.tile_pool(name="adj", bufs=adj_bufs) as apool,
            tc.tile_pool(name="xt", bufs=1) as xpool,
            tc.tile_pool(name="raw", bufs=2) as rpool,
            tc.tile_pool(name="post", bufs=4) as opool,
            tc.tile_pool(name="psr", bufs=4, space="PSUM") as ps_raw,
            tc.tile_pool(name="psd", bufs=2, space="PSUM") as ps_deg,
            tc.tile_pool(name="pso", bufs=2, space="PSUM") as ps_out,
        ):
            ps_warm = ps_deg  # warmup borrows a deg bank (PSUM is full)
            # PE warm-up: ~3 us of junk matmuls while the first adj chunks
            # stream in, so the HAM clock gate is at 2.4 GHz (not the cold
            # 1.2) when the real matmuls start.
            warm_sb = cpool.tile([P, CH], f16, tag="warm")
            nc.gpsimd.memset(warm_sb[:], 0)
            pw = ps_warm.tile([P, CH], f32, tag="psdeg")
            for i in range(7):
                nc.tensor.matmul(pw[:, :], warm_sb[:, 0:P], warm_sb[:, :],
                                 start=True, stop=True)

            # X + consts go on the Activation-engine HWDGE ring (X first:
            # it gates the first matmul) so the Sync ring's first dispatch
            # is already the first adj chunk and both rings stream in
            # parallel from t~7.5us.
            x_sb = xpool.tile([P, nbatch, KT, fin], f16, tag="x")
            # two halves: the first (batches 0-1) completes ~1.2us earlier
            # and is all the first batch's matmuls need
            nh = nbatch // 2
            nc.scalar.dma_start(x_sb[:, 0:nh], x[:, 0:nh])
            nc.scalar.dma_start(x_sb[:, nh:nbatch], x[:, nh:nbatch])
            w2_sb = cpool.tile([P, 2, P], f16, tag="w")
            nc.scalar.dma_start(w2_sb[:], w2[:, :, :])
            wT_sb = w2_sb[:, 0, :]
            onesW_sb = w2_sb[:, 1, :]
            b_sb = cpool.tile([P, 1], f32, tag="b")
            nc.scalar.dma_start(b_sb[:], bvec[:, :])
            if not use_lrelu:
                b2_sb = cpool.tile([P, 1], f32, tag="b2")
                nc.vector.tensor_scalar_mul(b2_sb[:], b_sb[:], 1.0 - alpha)
                b3_sb = cpool.tile([P, 1], f32, tag="b3")
                nc.vector.tensor_scalar_mul(b3_sb[:], b_sb[:], alpha)

            # Prefetch ALL adjacency DMAs up-front (16 x 512 KiB on the
            # Sync ring): the DMA engines stream continuously and the
            # per-chunk semaphores release compute at fine grain.
            all_chunks = []
            for b in range(nbatch):
                for c2 in range(NAC):
                    ac = apool.tile([P, KG, n], f16, tag="adj",
                                    name=f"ac{b}_{c2}")
                    nc.sync.dma_start(ac[:], adjT[b, c2])
                    all_chunks.append(ac)

            for b in range(nbatch):
                adj_chunks = all_chunks[NAC * b:NAC * b + NAC]

                def adj_slice(k, c):
                    return adj_chunks[k // KG][:, k % KG, c * CH:(c + 1) * CH]

                def aslc(k):
                    return adj_chunks[k // KG][:, k % KG, :]

                # deg: fold 8 k-tiles to 4 with pairwise DVE adds (fp16 2x);
                # accumulating ones-weights matmuls fold the rest and
                # broadcast deg to every output partition.
                def emit_deg():
                    pa = []
                    for g in range(KT // 2):
                        pt = rpool.tile([P, n], f16, tag=f"pa{g}")
                        nc.vector.tensor_tensor(
                            pt[:, :], aslc(2 * g), aslc(2 * g + 1),
                            mybir.AluOpType.add)
                        pa.append(pt)
                    # fold the last two pairs once more: 3 deg groups
                    # balances DVE adds (5) vs ones-matmuls (3/chunk)
                    pq = rpool.tile([P, n], f16, tag="pa45")
                    nc.vector.tensor_tensor(
                        pq[:, :], pa[2][:, :], pa[3][:, :],
                        mybir.AluOpType.add)
                    pa = [pa[0], pa[1], pq]
                    recs = []
                    for c in range(NCH):
                        ps_db = ps_deg.tile([P, CH], f32, tag="psdeg")
                        for g in range(len(pa)):
                            nc.tensor.matmul(
                                ps_db[:, :],
                                onesW_sb[:, :],
                                pa[g][:, c * CH:(c + 1) * CH],
                                start=(g == 0),
                                stop=(g == len(pa) - 1),
                            )
                        # 1/deg on the scalar engine (reciprocal LUT).
                        # bass refuses Reciprocal directly, so emit a Copy
                        # and flip the func.
                        rec_sb = opool.tile([P, CH], f32, tag="rec",
                                            name=f"rec{c}")
                        _ai = nc.scalar.activation(
                            rec_sb[:, :], ps_db[:, :],
                            mybir.ActivationFunctionType.Copy,
                            bias=0.0, scale=1.0)
                        _ai.ins.func = mybir.ActivationFunctionType.Reciprocal
                        recs.append(rec_sb)
                    return recs

                # rawT matmuls, one accumulation group per 512-chunk
                ps_chunks = [
                    ps_raw.tile([P, CH], f32, tag="psraw", name=f"psraw{cc}")
                    for cc in range(NCH)
                ]

                def emit_main(c):
                    for k in range(KT):
                        nc.tensor.matmul(
                            ps_chunks[c][:, :],
                            x_sb[:, b, k, :],
                            adj_slice(k, c),
                            start=(k == 0),
                            stop=(k == KT - 1),
                        )

                raw_sb = rpool.tile([P, n], f16, tag="raw")
                o_full = opool.tile([P, n], f16, tag="ofull")

                def emit_epi(c, rec_sb):
                    # out2T[o, m] = sum_f WT[f, o] * rawT[f, m]
                    ps_o = ps_out.tile([P, CH], f32, tag="psout")
                    nc.tensor.matmul(
                        ps_o[:, :],
                        wT_sb[:, :],
                        raw_sb[:, c * CH:(c + 1) * CH],
                        start=True,
                        stop=True,
                    )
                    # t = out2T / deg (fp16 out: faster 16-bit DVE modes
                    # downstream; ~2^-11 relative rounding, negligible)
                    t_sb = opool.tile([P, CH], f16, tag="t")
                    nc.vector.tensor_tensor(
                        t_sb[:, :], ps_o[:, :], rec_sb[:, :],
                        mybir.AluOpType.mult,
                    )
                    if use_lrelu:
                        # outT = Lrelu(t + b), negative slope alpha.
                        # (Unused by default: Lrelu lives in a different
                        # ACT table set than Reciprocal, and the per-batch
                        # ACT_TABLE_LOAD thrash costs ~10 us/core.)
                        nc.scalar.activation(
                            o_full[:, c * CH:(c + 1) * CH], t_sb[:, :],
                            mybir.ActivationFunctionType.Lrelu,
                            bias=b_sb[:, 0:1], scale=1.0, alpha=alpha,
                        )
                    else:
                        # u = alpha*t + alpha*b on the scalar engine
                        # (Identity is a filler function in every ACT
                        # table set, like Relu -- no table-set switch).
                        u_sb = opool.tile([P, CH], f16, tag="u")
                        nc.scalar.activation(
                            u_sb[:, :], t_sb[:, :],
                            mybir.ActivationFunctionType.Identity,
                            bias=b3_sb[:, 0:1], scale=alpha,
                        )
                        # r = Relu((1-a)*t + (1-a)*b) = (1-a)*Relu(t+b)
                        r_sb = opool.tile([P, CH], f16, tag="r")
                        nc.scalar.activation(
                            r_sb[:, :], t_sb[:, :],
                            mybir.ActivationFunctionType.Relu,
                            bias=b2_sb[:, 0:1], scale=1.0 - alpha,
                        )
                        # outT = u + r = leaky(t + b)
                        nc.vector.tensor_tensor(
                            o_full[:, c * CH:(c + 1) * CH], u_sb[:, :],
                            r_sb[:, :], mybir.AluOpType.add,
                        )
                    # store per chunk so the last batch's first half flies
                    # while the second half is still in the epilogue
                    nc.sync.dma_start(outT[b, :, c * CH:(c + 1) * CH],
                                      o_full[:, c * CH:(c + 1) * CH])

                # main matmuls first in the PE queue for every batch: any
                # other PE-order (deg first, chunk-serial with interleaved
                # W) stalls the PE on DVE/Act round-trips and loses far
                # more in the steady state than it saves in the tail.
                for k in range(KT):
                    for c in range(NCH):
                        nc.tensor.matmul(
                            ps_chunks[c][:, :],
                            x_sb[:, b, k, :],
                            adj_slice(k, c),
                            start=(k == 0),
                            stop=(k == KT - 1),
                        )
                # raw copies first in the Act FIFO: they feed the W/t
                # chain; the recs (which wait on the deg matmuls running
                # after the main block on the PE) queue behind them.
                for c in range(NCH):
                    nc.scalar.copy(raw_sb[:, c * CH:(c + 1) * CH],
                                   ps_chunks[c][:, :])
                recs = emit_deg()
                for c in range(NCH):
                    emit_epi(c, recs[c])

    _split_multi_waits(nc)
    return nc


def _split_multi_waits(nc):
    """Walrus rejects split-struct instructions with more than one sync wait
    ("Too many sync wait commands" in setupSyncWait<...>). Hoist all but the
    last wait of each multi-wait instruction onto same-engine no-ops inserted
    immediately before it (one wait per no-op)."""
    cnt = 0
    for f in nc.m.functions:
        for blk in f.blocks:
            idx = 0
            while idx < len(blk.instructions):
                inst = blk.instructions[idx]
                si = inst.sync_info
                if (type(inst).__name__ != "InstNoOp" and si is not None
                        and len(si.on_wait) > 1):
                    waits = list(si.on_wait)
                    for w in waits[:-1]:
                        nop = mybir.InstNoOp(name=f"mm_wait_nop_{cnt}",
                                             ins=[], outs=[])
                        cnt += 1
                        nop.engine = inst.engine
                        nop.sync_info = mybir.SyncInfo(on_wait=[w],
                                                       on_update=[])
                        nc.register_instruction(nop)
                        blk.instructions.insert(idx, nop)
                        idx += 1
                    inst.sync_info = mybir.SyncInfo(
                        on_wait=waits[-1:], on_update=list(si.on_update))
                idx += 1
    return cnt


_NC_CACHE = {}


def _get_nc():
    if "nc" not in _NC_CACHE:
        _NC_CACHE["nc"] = build_bass()
    return _NC_CACHE["nc"]


def _prep_in_maps(node_mat, adj_mat, W, b):
    node_mat = np.asarray(node_mat, dtype=np.float32)
    adj_mat = np.asarray(adj_mat, dtype=np.float32)
    wT = np.asarray(W, dtype=np.float32).T.astype(np.float16)
    w2 = np.ascontiguousarray(
        np.stack([wT, np.ones((P, P), np.float16)], axis=1))
    bvec = np.ascontiguousarray(
        np.asarray(b, dtype=np.float32).reshape(P, 1))
    NAC = 4
    KG = N // P // NAC
    in_maps = []
    for c in range(N_CORES):
        sl = slice(c * BPC, (c + 1) * BPC)
        # adjT_s[b, c2, p, g, m] = adj[b, m, c2*KG*P + g*P + p]
        adjT = np.ascontiguousarray(
            adj_mat[sl].astype(np.float16)
            .reshape(BPC, N, NAC, KG, P).transpose(0, 2, 4, 3, 1))
        # xs[p, b, k, f] = node[b, k*P + p, f]
        xs = np.ascontiguousarray(
            node_mat[sl].astype(np.float16)
            .reshape(BPC, N // P, P, FIN).transpose(2, 0, 1, 3))
        in_maps.append({
            "adjT": adjT,
            "x": xs,
            "w2": w2,
            "bvec": bvec,
        })
    return in_maps


def kernel(node_mat, adj_mat, W, b):
    nc = _get_nc()
    in_maps = _prep_in_maps(node_mat, adj_mat, W, b)
    res = run_bass_kernel_spmd(nc, in_maps, core_ids=list(range(N_CORES)))
    return np.ascontiguousarray(
        np.concatenate(
            [res.results[c]["outT"] for c in range(N_CORES)], axis=0
        ).swapaxes(1, 2).astype(np.float32)
    )


# revision 32
# speedup vs baseline: 1.0904x; 1.0904x over previous
"""GNN message-passing layer (normalized-adjacency conv + linear + LeakyReLU)
on 8 Trainium2 NeuronCores, pure data parallel over the batch dim.

Computation (per batch b):
    deg      = adj.sum(-1)                     # [N]
    agg      = (adj / deg[:, None]) @ X        # [N, FIN]
    out      = leakyrelu(agg @ W.T + bias)     # [N, FOUT]

Device-side formulation. adj is host-transposed per batch (adjT[k, m] =
adj[m, k]) so the contraction index k sits on SBUF partitions for both matmul
operands, and everything downstream stays transposed ([feature, node] order)
so all PE work streams 512-wide:
    rawT[f, m]   = sum_k X[k, f] * adjT[k, m]    # X tiles as weights
    degbc[:, m]  = sum_k 1 * adjT[k, m]          # ones weights -> deg
                                                 # broadcast to all partitions
    out2T[o, m]  = sum_f WT[f, o] * rawT[f, m]   # W as weights
    t            = out2T * (1/degbc)             # DVE multiply
    outT[o, m]   = Lrelu(t + b)                  # scalar engine, per-partition b
The DRAM output is [B, FOUT, N] fp16; the host swaps the last two axes and
casts to fp32.

Everything DMA'd is fp16 (half the HBM traffic of fp32; adj/X values are
well inside fp16 range and the 2^-11 rounding is far below the accuracy
gate). The deg reduction over the 8 k-tiles is split: 4 pairwise adds on the
DVE (fp16, 2x mode) fold 8 tiles to 4, then a 4-matmul PSUM accumulation
with ones weights folds the rest and broadcasts deg to all 128 partitions.
"""

import numpy as np

import concourse.bass as bass
import concourse.mybir as mybir
import concourse.tile as tile
from concourse.bass_utils import run_bass_kernel_spmd

P = 128

# Problem shape (hardcoded per the harness contract).
B, N, FIN, FOUT = 32, 1024, 128, 128
NEG_SLOPE = 0.01
N_CORES = 8
BPC = B // N_CORES  # batches per core

USE_LRELU = False


def build_bass(nbatch=BPC, n=N, fin=FIN, fout=FOUT, neg_slope=NEG_SLOPE,
               adj_bufs=8, use_lrelu=USE_LRELU):
    f32 = mybir.dt.float32
    f16 = mybir.dt.float16
    alpha = float(neg_slope)
    nc = bass.Bass()

    KT = n // P          # contraction tiles (8)
    CH = min(512, n)     # matmul moving free dim (one fp32 PSUM bank)
    NCH = n // CH        # moving-dim chunks (2)
    NAC = 4              # adj DMA chunks per batch (512 KiB each)
    KG = KT // NAC       # k-tiles per adj DMA chunk (2)

    # All DRAM layouts are host-staged so every DMA is one fully linear
    # run per partition (128 descriptors/transfer instead of 512+): the
    # Sync-engine HWDGE dispatch cost is descriptor-bound.
    adjT = nc.dram_tensor("adjT", [nbatch, NAC, P, KG, n], f16,
                          kind="ExternalInput")
    x = nc.dram_tensor("x", [P, nbatch, n // P, fin], f16,
                       kind="ExternalInput")
    w2 = nc.dram_tensor("w2", [P, 2, P], f16, kind="ExternalInput")
    bvec = nc.dram_tensor("bvec", [P, 1], f32, kind="ExternalInput")
    outT = nc.dram_tensor("outT", [nbatch, fout, n], f16, kind="ExternalOutput")

    with tile.TileContext(nc) as tc:
        with (
            tc.tile_pool(name="const", bufs=1) as cpool,
            tc.tile_pool(name="adj", bufs=adj_bufs) as apool,
            tc.tile_pool(name="xt", bufs=1) as xpool,
            tc.tile_pool(name="raw", bufs=2) as rpool,
            tc.tile_pool(name="post", bufs=4) as opool,
            tc.tile_pool(name="psr", bufs=4, space="PSUM") as ps_raw,
            tc.tile_pool(name="psd", bufs=2, space="PSUM") as ps_deg,
            tc.tile_pool(name="pso", bufs=2, space="PSUM") as ps_out,
        ):
            ps_warm = ps_deg  # warmup borrows a deg bank (PSUM is full)
            # PE warm-up: ~3 us of junk matmuls while the first adj chunks
            # stream in, so the HAM clock gate is at 2.4 GHz (not the cold
            # 1.2) when the real matmuls start.
            warm_sb = cpool.tile([P, CH], f16, tag="warm")
            nc.gpsimd.memset(warm_sb[:], 0)
            pw = ps_warm.tile([P, CH], f32, tag="psdeg")
            for i in range(7):
                nc.tensor.matmul(pw[:, :], warm_sb[:, 0:P], warm_sb[:, :],
                                 start=True, stop=True)

            # X + consts go on the Activation-engine HWDGE ring (X first:
            # it gates the first matmul) so the Sync ring's first dispatch
            # is already the first adj chunk and both rings stream in
            # parallel from t~7.5us.
            x_sb = xpool.tile([P, nbatch, KT, fin], f16, tag="x")
            nc.scalar.dma_start(x_sb[:], x[:, :, :, :])
            w2_sb = cpool.tile([P, 2, P], f16, tag="w")
            nc.scalar.dma_start(w2_sb[:], w2[:, :, :])
            wT_sb = w2_sb[:, 0, :]
            onesW_sb = w2_sb[:, 1, :]
            b_sb = cpool.tile([P, 1], f32, tag="b")
            nc.scalar.dma_start(b_sb[:], bvec[:, :])
            if not use_lrelu:
                b2_sb = cpool.tile([P, 1], f32, tag="b2")
                nc.vector.tensor_scalar_mul(b2_sb[:], b_sb[:], 1.0 - alpha)
                b3_sb = cpool.tile([P, 1], f32, tag="b3")
                nc.vector.tensor_scalar_mul(b3_sb[:], b_sb[:], alpha)

            # Prefetch ALL adjacency DMAs up-front (16 x 512 KiB on the
            # Sync ring): the DMA engines stream continuously and the
            # per-chunk semaphores release compute at fine grain.
            all_chunks = []
            for b in range(nbatch):
                for c2 in range(NAC):
                    ac = apool.tile([P, KG, n], f16, tag="adj",
                                    name=f"ac{b}_{c2}")
                    nc.sync.dma_start(ac[:], adjT[b, c2])
                    all_chunks.append(ac)

            for b in range(nbatch):
                adj_chunks = all_chunks[NAC * b:NAC * b + NAC]

                def adj_slice(k, c):
                    return adj_chunks[k // KG][:, k % KG, c * CH:(c + 1) * CH]

                def aslc(k):
                    return adj_chunks[k // KG][:, k % KG, :]

                # deg: fold 8 k-tiles to 4 with pairwise DVE adds (fp16 2x);
                # accumulating ones-weights matmuls fold the rest and
                # broadcast deg to every output partition.
                def emit_deg():
                    pa = []
                    for g in range(KT // 2):
                        pt = rpool.tile([P, n], f16, tag=f"pa{g}")
                        nc.vector.tensor_tensor(
                            pt[:, :], aslc(2 * g), aslc(2 * g + 1),
                            mybir.AluOpType.add)
                        pa.append(pt)
                    # fold the last two pairs once more: 3 deg groups
                    # balances DVE adds (5) vs ones-matmuls (3/chunk)
                    pq = rpool.tile([P, n], f16, tag="pa45")
                    nc.vector.tensor_tensor(
                        pq[:, :], pa[2][:, :], pa[3][:, :],
                        mybir.AluOpType.add)
                    pa = [pa[0], pa[1], pq]
                    recs = []
                    for c in range(NCH):
                        ps_db = ps_deg.tile([P, CH], f32, tag="psdeg")
                        for g in range(len(pa)):
                            nc.tensor.matmul(
                                ps_db[:, :],
                                onesW_sb[:, :],
                                pa[g][:, c * CH:(c + 1) * CH],
                                start=(g == 0),
                                stop=(g == len(pa) - 1),
                            )
                        # 1/deg on the scalar engine (reciprocal LUT).
                        # bass refuses Reciprocal directly, so emit a Copy
                        # and flip the func.
                        rec_sb = opool.tile([P, CH], f32, tag="rec",
                                            name=f"rec{c}")
                        _ai = nc.scalar.activation(
                            rec_sb[:, :], ps_db[:, :],
                            mybir.ActivationFunctionType.Copy,
                            bias=0.0, scale=1.0)
                        _ai.ins.func = mybir.ActivationFunctionType.Reciprocal
                        recs.append(rec_sb)
                    return recs

                # rawT matmuls, one accumulation group per 512-chunk
                ps_chunks = [
                    ps_raw.tile([P, CH], f32, tag="psraw", name=f"psraw{cc}")
                    for cc in range(NCH)
                ]

                def emit_main(c):
                    for k in range(KT):
                        nc.tensor.matmul(
                            ps_chunks[c][:, :],
                            x_sb[:, b, k, :],
                            adj_slice(k, c),
                            start=(k == 0),
                            stop=(k == KT - 1),
                        )

                raw_sb = rpool.tile([P, n], f16, tag="raw")
                o_full = opool.tile([P, n], f16, tag="ofull")

                def emit_epi(c, rec_sb):
                    # out2T[o, m] = sum_f WT[f, o] * rawT[f, m]
                    ps_o = ps_out.tile([P, CH], f32, tag="psout")
                    nc.tensor.matmul(
                        ps_o[:, :],
                        wT_sb[:, :],
                        raw_sb[:, c * CH:(c + 1) * CH],
                        start=True,
                        stop=True,
                    )
                    # t = out2T / deg (fp16 out: faster 16-bit DVE modes
                    # downstream; ~2^-11 relative rounding, negligible)
                    t_sb = opool.tile([P, CH], f16, tag="t")
                    nc.vector.tensor_tensor(
                        t_sb[:, :], ps_o[:, :], rec_sb[:, :],
                        mybir.AluOpType.mult,
                    )
                    if use_lrelu:
                        # outT = Lrelu(t + b), negative slope alpha.
                        # (Unused by default: Lrelu lives in a different
                        # ACT table set than Reciprocal, and the per-batch
                        # ACT_TABLE_LOAD thrash costs ~10 us/core.)
                        nc.scalar.activation(
                            o_full[:, c * CH:(c + 1) * CH], t_sb[:, :],
                            mybir.ActivationFunctionType.Lrelu,
                            bias=b_sb[:, 0:1], scale=1.0, alpha=alpha,
                        )
                    else:
                        # u = alpha*t + alpha*b on the scalar engine
                        # (Identity is a filler function in every ACT
                        # table set, like Relu -- no table-set switch).
                        u_sb = opool.tile([P, CH], f16, tag="u")
                        nc.scalar.activation(
                            u_sb[:, :], t_sb[:, :],
                            mybir.ActivationFunctionType.Identity,
                            bias=b3_sb[:, 0:1], scale=alpha,
                        )
                        # r = Relu((1-a)*t + (1-a)*b) = (1-a)*Relu(t+b)
                        r_sb = opool.tile([P, CH], f16, tag="r")
                        nc.scalar.activation(
                            r_sb[:, :], t_sb[:, :],
                            mybir.ActivationFunctionType.Relu,
                            bias=b2_sb[:, 0:1], scale=1.0 - alpha,
                        )
                        # outT = u + r = leaky(t + b)
                        nc.vector.tensor_tensor(
                            o_full[:, c * CH:(c + 1) * CH], u_sb[:, :],
                            r_sb[:, :], mybir.AluOpType.add,
                        )
                    # store per chunk so the last batch's first half flies
                    # while the second half is still in the epilogue
                    nc.sync.dma_start(outT[b, :, c * CH:(c + 1) * CH],
                                      o_full[:, c * CH:(c + 1) * CH])

                # main matmuls first in the PE queue for every batch: any
                # other PE-order (deg first, chunk-serial with interleaved
                # W) stalls the PE on DVE/Act round-trips and loses far
                # more in the steady state than it saves in the tail.
                for k in range(KT):
                    for c in range(NCH):
                        nc.tensor.matmul(
                            ps_chunks[c][:, :],
                            x_sb[:, b, k, :],
                            adj_slice(k, c),
                            start=(k == 0),
                            stop=(k == KT - 1),
                        )
                # raw copies first in the Act FIFO: they feed the W/t
                # chain; the recs (which wait on the deg matmuls running
                # after the main block on the PE) queue behind them.
                for c in range(NCH):
                    nc.scalar.copy(raw_sb[:, c * CH:(c + 1) * CH],
                                   ps_chunks[c][:, :])
                recs = emit_deg()
                for c in range(NCH):
                    emit_epi(c, recs[c])

    _split_multi_waits(nc)
    return nc


def _split_multi_waits(nc):
    """Walrus rejects split-struct instructions with more than one sync wait
    ("Too many sync wait commands" in setupSyncWait<...>). Hoist all but the
    last wait of each multi-wait instruction onto same-engine no-ops inserted
    immediately before it (one wait per no-op)."""
    cnt = 0
    for f in nc.m.functions:
        for blk in f.blocks:
            idx = 0
            while idx < len(blk.instructions):
                inst = blk.instructions[idx]
                si = inst.sync_info
                if (type(inst).__name__ != "InstNoOp" and si is not None
                        and len(si.on_wait) > 1):
                    waits = list(si.on_wait)
                    for w in waits[:-1]:
                        nop = mybir.InstNoOp(name=f"mm_wait_nop_{cnt}",
                                             ins=[], outs=[])
                        cnt += 1
                        nop.engine = inst.engine
                        nop.sync_info = mybir.SyncInfo(on_wait=[w],
                                                       on_update=[])
                        nc.register_instruction(nop)
                        blk.instructions.insert(idx, nop)
                        idx += 1
                    inst.sync_info = mybir.SyncInfo(
                        on_wait=waits[-1:], on_update=list(si.on_update))
                idx += 1
    return cnt


_NC_CACHE = {}


def _get_nc():
    if "nc" not in _NC_CACHE:
        _NC_CACHE["nc"] = build_bass()
    return _NC_CACHE["nc"]


def _prep_in_maps(node_mat, adj_mat, W, b):
    node_mat = np.asarray(node_mat, dtype=np.float32)
    adj_mat = np.asarray(adj_mat, dtype=np.float32)
    wT = np.asarray(W, dtype=np.float32).T.astype(np.float16)
    w2 = np.ascontiguousarray(
        np.stack([wT, np.ones((P, P), np.float16)], axis=1))
    bvec = np.ascontiguousarray(
        np.asarray(b, dtype=np.float32).reshape(P, 1))
    NAC = 4
    KG = N // P // NAC
    in_maps = []
    for c in range(N_CORES):
        sl = slice(c * BPC, (c + 1) * BPC)
        # adjT_s[b, c2, p, g, m] = adj[b, m, c2*KG*P + g*P + p]
        adjT = np.ascontiguousarray(
            adj_mat[sl].astype(np.float16)
            .reshape(BPC, N, NAC, KG, P).transpose(0, 2, 4, 3, 1))
        # xs[p, b, k, f] = node[b, k*P + p, f]
        xs = np.ascontiguousarray(
            node_mat[sl].astype(np.float16)
            .reshape(BPC, N // P, P, FIN).transpose(2, 0, 1, 3))
        in_maps.append({
            "adjT": adjT,
            "x": xs,
            "w2": w2,
            "bvec": bvec,
        })
    return in_maps


def kernel(node_mat, adj_mat, W, b):
    nc = _get_nc()
    in_maps = _prep_in_maps(node_mat, adj_mat, W, b)
    res = run_bass_kernel_spmd(nc, in_maps, core_ids=list(range(N_CORES)))
    return np.ascontiguousarray(
        np.concatenate(
            [res.results[c]["outT"] for c in range(N_CORES)], axis=0
        ).swapaxes(1, 2).astype(np.float32)
    )
